# revision 17
# baseline (speedup 1.0000x reference)
"""GAT (2-layer, DGL-style) on 8 Trainium2 NeuronCores — v4.

v1 bottleneck (profiled): layer-2's per-edge dma_gather is ~6.5ns/idx of
Q7 descriptor-generation, serialized AFTER layer 1 + a monolithic
AllGather barrier -> a 1.15ms tail where only GPSIMD works.

v4 structure:
  - Layer 1 unchanged mathematically (edge-partitioned by dst tile,
    gather-free, one-hot mask matmuls).  Diets: er-of-own-nodes via a
    host-folded weight (kills a 256-wide matmul + reduce per slot), and
    the PSUM z is copied to SBUF bf16 on the ACT engine so the big
    ex-multiply runs bf16xbf16 on DVE at 2x rate (DVE was the busiest
    engine in L1).
  - The 49 dst slots are split into 8 groups (7,7,7,7,7,6,5,3).  When a
    group's z2 rows (z2|el2|er2 bit-packed bf16) finish, an AllGather
    for just that group fires and layer-2 dma_gather calls for edges
    whose SRC is stored in that group start — overlapping the rest of
    layer 1 on the otherwise-idle GPSIMD engine.  Each group AllGather
    reads a dedicated FULL local tensor (z2locs[g]) — a sliced
    collective input raced with its writers on cold runs.
  - Layer-2 consumption runs piece-wise (16-chunk pieces of each group
    stream), gated on the group's gather emission being >= CONSUME_LAG
    slots old AND on er2 of every dst slot the piece touches being
    computed (else the consumption ops camp at the DVE/PE queue heads
    and stall layer 1); results accumulate into an SBUF [128,49,41] f32
    accumulator, divided + biased at the end.
"""

import heapq
import sys

sys.path.insert(0, "/opt/trn_rl_repo")

import ml_dtypes
import numpy as np

import concourse.bacc as bacc
import concourse.tile as tile
from concourse import mybir
from concourse.bass_utils import run_bass_kernel_spmd

F32 = mybir.dt.float32
BF16 = mybir.dt.bfloat16
I16 = mybir.dt.int16
NP_BF16 = ml_dtypes.bfloat16

# Problem constants (hardcoded per contract)
N_NODES = 50000
N_EDGES = 800000
IN_SIZE = 128
HID = 32
H1 = 8
HD1 = H1 * HID  # 256
OUT = 40
H2 = 1
NEG_SLOPE = 0.2

N_CORES = 8
P = 128
N_PAD = 50176  # 392 * 128
TILES = N_PAD // P  # 392
TPC = TILES // N_CORES  # 49 slots per core
NPC = N_PAD // N_CORES  # 6272 nodes per core

W1A = HD1 + H1  # 264: layer-1 per-edge matmul out [z 256 | el 8]
Z2ROW = 128  # bf16: [z2 40 | el2 f32 (2 slots) | er2 bf16 (1 slot) | pad]
AGG2_W = OUT + 1  # 41 rhs cols
MAX_GIDX = 1024  # max indices per dma_gather call (q7 scratch limit)
ZB = 2  # layer-1 chunks staged in PSUM per batch
PIECE_CH = 16  # layer-2 consumption piece size in 128-edge chunks
INFLIGHT_CAP = 4  # gathered-not-yet-consumed pieces (< zg2 pool bufs)
CONSUME_LAG = 2  # slots between a piece's gather emission and consumption
EPS = 1e-30

# slot groups for the pipelined AllGather (sum 49; smaller at the end so
# the exposed post-L1 gather tail is short)
GROUP_SIZES = [7, 7, 7, 7, 7, 6, 5, 3]
N_G = len(GROUP_SIZES)
GSTART = np.concatenate([[0], np.cumsum(GROUP_SIZES)]).astype(np.int64)


# --------------------------------------------------------------------------
# Host-side plan
# --------------------------------------------------------------------------
def _pack_idxs(vals):
    """dma_gather idx layout: [128, n/16] int16, idx i at [i%16, i//16],
    replicated across the 8 q7 core pairs."""
    n = len(vals)
    assert n % 16 == 0
    arr = np.asarray(vals, np.int16).reshape(n // 16, 16).T  # [16, n/16]
    return np.tile(arr, (8, 1))


def build_plan(src, dst):
    src = np.asarray(src).astype(np.int64)
    dst = np.asarray(dst).astype(np.int64)
    order = np.argsort(dst, kind="stable")
    src_s = src[order]
    dst_s = dst[order]
    tile_of = dst_s // P
    counts = np.bincount(tile_of, minlength=TILES)
    starts = np.zeros(TILES + 1, np.int64)
    starts[1:] = np.cumsum(counts)

    # Balanced tile -> (core, slot) assignment (snake order over slots).
    tiles_sorted = sorted(range(TILES), key=lambda t: -counts[t])
    tile_at = {}
    idx = 0
    for j in range(TPC):
        ks = range(N_CORES) if j % 2 == 0 else range(N_CORES - 1, -1, -1)
        for k in ks:
            tile_at[(k, j)] = tiles_sorted[idx]
            idx += 1

    pi = np.empty(N_PAD, np.int64)
    for (k, j), t in tile_at.items():
        pi[t * P : (t + 1) * P] = k * NPC + j * P + np.arange(P)

    # ---- Layer-1 plan ----
    cnt1 = np.zeros((N_CORES, TPC), np.int64)
    edges1 = {}
    for (k, j), t in tile_at.items():
        st, en = starts[t], starts[t + 1]
        edges1[(k, j)] = (src_s[st:en], dst_s[st:en])
        cnt1[k, j] = en - st
    cap1 = ((cnt1.max(axis=0) + P - 1) // P) * P
    m_j = cap1 // P
    M = int(m_j.sum())

    src_e = np.full((N_CORES, M * P), N_PAD, np.int64)  # N_PAD = pad row
    row_e = np.full((N_CORES, M * P), -1, np.int64)
    moff = np.zeros(TPC + 1, np.int64)
    moff[1:] = np.cumsum(m_j)
    for k in range(N_CORES):
        for j in range(TPC):
            s, d = edges1[(k, j)]
            t = tile_at[(k, j)]
            off = int(moff[j]) * P
            src_e[k, off : off + len(s)] = s
            row_e[k, off : off + len(s)] = d - t * P

    # ---- Layer-2 plan ----
    slot_of_pos = (pi % NPC) // P
    rank_of_pos = pi // NPC
    g_of_slot = np.zeros(TPC, np.int64)
    for g in range(N_G):
        g_of_slot[GSTART[g] : GSTART[g + 1]] = g
    node_g = g_of_slot[slot_of_pos]  # [N_PAD] src group of each node
    node_grow = (
        rank_of_pos * np.array(GROUP_SIZES, dtype=np.int64)[node_g]
        + (slot_of_pos - GSTART[node_g])
    ) * P + (pi % P)

    seg_edges = {}
    seg_cnt = np.zeros((N_CORES, N_G, TPC), np.int64)
    for k in range(N_CORES):
        for j in range(TPC):
            s, d = edges1[(k, j)]
            t = tile_at[(k, j)]
            gs = node_g[s]
            o2 = np.argsort(gs, kind="stable")
            s2, d2, gs2 = s[o2], d[o2], gs[o2]
            for g in range(N_G):
                m = gs2 == g
                seg_edges[(k, g, j)] = (s2[m], d2[m] - t * P)
                seg_cnt[k, g, j] = int(m.sum())
    seg_len = ((seg_cnt.max(axis=0) + 15) // 16) * 16  # [N_G, TPC]
    glen = seg_len.sum(axis=1)
    gpad = ((glen + P - 1) // P) * P - glen
    seg_len[:, TPC - 1] += gpad
    glen = seg_len.sum(axis=1)
    assert (glen % P == 0).all()
    gchunks = glen // P
    goff = np.zeros(N_G + 1, np.int64)
    goff[1:] = np.cumsum(glen)
    tot_idx = int(goff[-1])

    zidx = np.zeros((N_CORES, tot_idx), np.int64)
    row2 = np.full((N_CORES, tot_idx), -1, np.int64)
    slot2 = np.full(tot_idx, -1, np.int64)
    pos = 0
    for g in range(N_G):
        for j in range(TPC):
            slot2[pos : pos + seg_len[g, j]] = j
            for k in range(N_CORES):
                s, r = seg_edges[(k, g, j)]
                zidx[k, pos : pos + len(s)] = node_grow[s]
                row2[k, pos : pos + len(s)] = r
            pos += int(seg_len[g, j])
    assert pos == tot_idx

    # chunk -> list of (slot, a, b) touches
    tot_chunks = int(gchunks.sum())
    touches = []
    for c in range(tot_chunks):
        sl = slot2[c * P : (c + 1) * P]
        tl = []
        a = 0
        while a < P:
            j = int(sl[a])
            b = a
            while b < P and sl[b] == j:
                b += 1
            tl.append((j, a, b))
            a = b
        touches.append(tl)

    return dict(
        m_j=[int(x) for x in m_j],
        M=M,
        pi=pi,
        src_e=src_e,
        row_e=row_e,
        seg_len=seg_len,
        glen=[int(x) for x in glen],
        goff=[int(x) for x in goff],
        gchunks=[int(x) for x in gchunks],
        touches=touches,
        zidx=zidx,
        row2=row2,
        slot2=slot2,
        tot_idx=tot_idx,
    )


# --------------------------------------------------------------------------
# Device program
# --------------------------------------------------------------------------
def build_program(plan):
    m_j, M = plan["m_j"], plan["M"]
    glen, goff, gchunks = plan["glen"], plan["goff"], plan["gchunks"]
    touches = plan["touches"]
    slot2 = plan["slot2"]
    tot_idx = plan["tot_idx"]
    tot_chunks = sum(gchunks)

    # chunk index of each group's first chunk
    gch0 = np.zeros(N_G + 1, np.int64)
    gch0[1:] = np.cumsum(gchunks)
    # touch -> mask column block index
    touch_base = np.zeros(tot_chunks + 1, np.int64)
    for c in range(tot_chunks):
        touch_base[c + 1] = touch_base[c] + len(touches[c])
    n_touch = int(touch_base[-1])

    # pieces: (g, pc) -> chunk range [c0, c1), j_last, calls
    pieces = []
    for g in range(N_G):
        npc_g = (gchunks[g] + PIECE_CH - 1) // PIECE_CH
        for pc in range(npc_g):
            c0 = pc * PIECE_CH
            c1 = min(c0 + PIECE_CH, gchunks[g])
            base = int(goff[g]) + c0 * P
            nidx = (c1 - c0) * P
            calls = []
            q = 0
            while q < nidx:
                n = min(MAX_GIDX, nidx - q)
                calls.append((base + q, q // P, n))
                q += n
            j_last = int(slot2[base + nidx - 1])
            pieces.append(dict(g=g, pc=pc, c0=c0, c1=c1, calls=calls,
                               j_last=j_last))
    piece_of = {(p["g"], p["pc"]): p for p in pieces}
    maxt = max(
        int(touch_base[int(gch0[p["g"]]) + p["c1"]]
            - touch_base[int(gch0[p["g"]]) + p["c0"]])
        for p in pieces
    )

    nc = bacc.Bacc(
        "TRN2",
        target_bir_lowering=False,
        debug=False,
        enable_asserts=False,
        num_devices=N_CORES,
    )

    feT = nc.dram_tensor("feT", [P, M * P], BF16, kind="ExternalInput").ap()
    maskh = nc.dram_tensor("maskh", [P, M * P], BF16, kind="ExternalInput").ap()
    maskth = nc.dram_tensor("maskth", [P, M * P], BF16, kind="ExternalInput").ap()
    featT = nc.dram_tensor("featT", [P, NPC], BF16, kind="ExternalInput").ap()
    w1aug = nc.dram_tensor("w1aug", [P, W1A], BF16, kind="ExternalInput").ap()
    w1 = nc.dram_tensor("w1", [P, HD1], BF16, kind="ExternalInput").ap()
    wer1 = nc.dram_tensor("wer1", [P, H1], BF16, kind="ExternalInput").ap()
    b1m = nc.dram_tensor("b1m", [P, HD1], F32, kind="ExternalInput").ap()
    w2a = nc.dram_tensor("w2a", [P, OUT], F32, kind="ExternalInput").ap()
    w2b = nc.dram_tensor("w2b", [P, OUT], F32, kind="ExternalInput").ap()
    al2m = nc.dram_tensor("al2m", [P, OUT], F32, kind="ExternalInput").ap()
    ar2m = nc.dram_tensor("ar2m", [P, OUT], F32, kind="ExternalInput").ap()
    b2m = nc.dram_tensor("b2m", [P, OUT], F32, kind="ExternalInput").ap()
    mask2h = nc.dram_tensor(
        "mask2h", [P, n_touch * P], BF16, kind="ExternalInput"
    ).ap()
    mask2t = nc.dram_tensor(
        "mask2t", [P, n_touch * P], BF16, kind="ExternalInput"
    ).ap()
    zidx_d = nc.dram_tensor(
        "zidx", [P, tot_idx // 16], I16, kind="ExternalInput"
    ).ap()
    out_d = nc.dram_tensor("out", [NPC, OUT], F32, kind="ExternalOutput").ap()

    moff = np.zeros(TPC + 1, np.int64)
    moff[1:] = np.cumsum(m_j)
    g_of_slot_py = np.zeros(TPC, np.int64)
    for g in range(N_G):
        g_of_slot_py[GSTART[g] : GSTART[g + 1]] = g

    with tile.TileContext(nc) as tc:
        with (
            tc.tile_pool(name="const", bufs=1) as cpool,
            tc.tile_pool(name="dram", bufs=1, space="DRAM") as dpool,
        ):
            w1aug_s = cpool.tile([P, W1A], BF16)
            w1_s = cpool.tile([P, HD1], BF16)
            wer1_s = cpool.tile([P, H1], BF16)
            b1_s = cpool.tile([P, HD1], F32)
            w2a_s = cpool.tile([P, OUT], F32)
            w2b_s = cpool.tile([P, OUT], F32)
            al2_s = cpool.tile([P, OUT], F32)
            ar2_s = cpool.tile([P, OUT], F32)
            b2_s = cpool.tile([P, OUT], F32)
            zidx_s = cpool.tile([P, tot_idx // 16], I16)
            ident = cpool.tile([P, P], F32)
            er2all = cpool.tile([P, TPC], BF16)
            acc_all = cpool.tile([P, TPC * AGG2_W], F32)
            from concourse.masks import make_identity

            for sb, dr in [
                (w1aug_s, w1aug), (w1_s, w1), (wer1_s, wer1), (b1_s, b1m),
                (w2a_s, w2a), (w2b_s, w2b), (al2_s, al2m), (ar2_s, ar2m),
                (b2_s, b2m), (zidx_s, zidx_d),
            ]:
                nc.sync.dma_start(out=sb[:], in_=dr)
            make_identity(nc, ident[:])
            nc.vector.memset(acc_all[:], 0.0)
            acc3 = acc_all[:].rearrange("p (j w) -> p j w", w=AGG2_W)

            # per-group local z2 rows (full-tensor AllGather inputs, so
            # the collective's input-writer annotation sees every slot
            # DMA; a sliced input raced on cold runs)
            z2locs = [
                dpool.tile(
                    [GROUP_SIZES[g] * P, Z2ROW],
                    BF16,
                    name=f"z2loc_g{g}",
                )
                for g in range(N_G)
            ]
            z2tabs = [
                dpool.tile(
                    [N_CORES * GROUP_SIZES[g] * P, Z2ROW],
                    BF16,
                    addr_space="Shared",
                    name=f"z2tab_g{g}",
                )
                for g in range(N_G)
            ]

            with (
                tc.tile_pool(name="l1", bufs=3) as lp,
                tc.tile_pool(name="l1_sm", bufs=4) as sm1,
                tc.tile_pool(name="l1_zps", bufs=2, space="PSUM") as pzp,
                tc.tile_pool(name="l1_acc", bufs=2, space="PSUM") as pac,
                tc.tile_pool(name="l2", bufs=5) as ap2,
                tc.tile_pool(name="l2_sm", bufs=4) as sm2,
                tc.tile_pool(name="l2_ps", bufs=2, space="PSUM") as pp3,
            ):
                # ---------- L2 emission machinery ----------
                gather_heap = []  # (j_last, g, pc)
                gathered = []  # (key, emit_slot) gathered, to consume
                zg_bufs = {}

                def emit_piece_gather(key, j_now):
                    p = piece_of[key]
                    g = p["g"]
                    buf = ap2.tile([P, PIECE_CH, Z2ROW], BF16, tag="zg2")
                    zg_bufs[key] = buf
                    for (p0, crel, n) in p["calls"]:
                        nc.gpsimd.dma_gather(
                            buf[:, crel : crel + n // P, :],
                            z2tabs[g][:],
                            zidx_s[:, p0 // 16 : (p0 + n) // 16],
                            n, n, Z2ROW,
                        )
                    gathered.append((key, j_now))

                def consume_piece(key):
                    p = piece_of[key]
                    g, c0p, c1p = p["g"], p["c0"], p["c1"]
                    nch = c1p - c0p
                    ch0 = int(gch0[g]) + c0p  # global chunk index
                    tb0 = int(touch_base[ch0])
                    ntp = int(touch_base[ch0 + nch]) - tb0
                    buf = zg_bufs.pop(key)
                    # batched mask loads for the whole piece
                    mh_sl = ap2.tile([P, maxt, P], BF16, tag="mh")
                    nc.sync.dma_start(
                        out=mh_sl[:, 0:ntp, :],
                        in_=mask2h[:, tb0 * P : (tb0 + ntp) * P],
                    )
                    mt_sl = ap2.tile([P, maxt, P], BF16, tag="mt")
                    nc.sync.dma_start(
                        out=mt_sl[:, 0:ntp, :],
                        in_=mask2t[:, tb0 * P : (tb0 + ntp) * P],
                    )
                    l2ps = pp3.tile([P, 512], F32, tag="l2ps")
                    erp = l2ps[:, 128 : 128 + PIECE_CH]
                    # er2[dst] per edge
                    for ci in range(nch):
                        tl = touches[ch0 + ci]
                        tb = int(touch_base[ch0 + ci]) - tb0
                        for ti, (j, a, b) in enumerate(tl):
                            nc.tensor.matmul(
                                out=erp[:, ci : ci + 1],
                                lhsT=mt_sl[:, tb + ti, :],
                                rhs=er2all[:, j : j + 1],
                                start=(ti == 0),
                                stop=(ti == len(tl) - 1),
                            )
                    # ev2 = el2 + er2 ; ex2 = max(exp, exp(0.2*))
                    ev2 = sm2.tile([P, PIECE_CH], F32, tag="ev2")
                    nc.vector.tensor_tensor(
                        out=ev2[:, 0:nch].unsqueeze(2),
                        in0=buf[:, 0:nch, 40:42].bitcast(F32),
                        in1=erp[:, 0:nch].unsqueeze(2),
                        op=mybir.AluOpType.add,
                    )
                    ex2a = sm2.tile([P, PIECE_CH], F32, tag="ex2a")
                    nc.scalar.activation(
                        out=ex2a[:, 0:nch], in_=ev2[:, 0:nch],
                        func=mybir.ActivationFunctionType.Exp,
                    )
                    ex2b = sm2.tile([P, PIECE_CH], F32, tag="ex2b")
                    nc.scalar.activation(
                        out=ex2b[:, 0:nch], in_=ev2[:, 0:nch],
                        func=mybir.ActivationFunctionType.Exp,
                        scale=NEG_SLOPE,
                    )
                    ex2 = sm2.tile([P, PIECE_CH], BF16, tag="ex2")
                    nc.vector.tensor_tensor(
                        out=ex2[:, 0:nch], in0=ex2a[:, 0:nch],
                        in1=ex2b[:, 0:nch], op=mybir.AluOpType.max,
                    )
                    zs2 = ap2.tile([P, PIECE_CH, AGG2_W], BF16, tag="zs2")
                    nc.vector.tensor_tensor(
                        out=zs2[:, 0:nch, 0:OUT],
                        in0=buf[:, 0:nch, 0:OUT],
                        in1=ex2[:, 0:nch].unsqueeze(2).to_broadcast(
                            [P, nch, OUT]
                        ),
                        op=mybir.AluOpType.mult,
                    )
                    nc.vector.tensor_copy(
                        out=zs2[:, 0:nch, OUT : OUT + 1],
                        in_=ex2[:, 0:nch].unsqueeze(2),
                    )
                    # scatter: group consecutive touches by slot
                    flat = []
                    for ci in range(nch):
                        tl = touches[ch0 + ci]
                        tb = int(touch_base[ch0 + ci]) - tb0
                        for ti, (j, a, b) in enumerate(tl):
                            flat.append((j, ci, tb + ti))
                    runs = []
                    for (j, ci, tcol) in flat:
                        if runs and runs[-1][0] == j:
                            runs[-1][1].append((ci, tcol))
                        else:
                            runs.append((j, [(ci, tcol)]))
                    for ri, (j, items) in enumerate(runs):
                        acc2 = l2ps[:, 64 * (ri % 2) : 64 * (ri % 2) + AGG2_W]
                        for qi, (ci, tcol) in enumerate(items):
                            nc.tensor.matmul(
                                out=acc2,
                                lhsT=mh_sl[:, tcol, :],
                                rhs=zs2[:, ci, :],
                                start=(qi == 0),
                                stop=(qi == len(items) - 1),
                            )
                        nc.vector.tensor_tensor(
                            out=acc3[:, j, :],
                            in0=acc3[:, j, :],
                            in1=acc2,
                            op=mybir.AluOpType.add,
                        )

                def pump(j_now, consume_budget=2, drain=False):
                    # start gathers for pieces (soonest-consumable first)
                    while gather_heap and len(zg_bufs) < INFLIGHT_CAP:
                        _, g, pc = heapq.heappop(gather_heap)
                        emit_piece_gather((g, pc), j_now)
                    # consume eligible pieces (er2 of all touched slots
                    # written, and gathers had CONSUME_LAG slots to land)
                    done = 0
                    i = 0
                    while i < len(gathered) and done < consume_budget:
                        key, es = gathered[i]
                        if piece_of[key]["j_last"] <= j_now - 1 and (
                            drain or es + CONSUME_LAG <= j_now
                        ):
                            gathered.pop(i)
                            consume_piece(key)
                            done += 1
                        else:
                            i += 1

                # ---------- main loop: L1 slots + interleaved L2 ----------
                g_next = 0
                for j in range(TPC):
                    m = m_j[j]
                    c0 = int(moff[j])
                    feT_sl = lp.tile([P, m, P], BF16, tag="feT")
                    nc.sync.dma_start(
                        out=feT_sl[:], in_=feT[:, c0 * P : (c0 + m) * P]
                    )
                    mask_sl = lp.tile([P, m, P], BF16, tag="mask")
                    nc.sync.dma_start(
                        out=mask_sl[:], in_=maskh[:, c0 * P : (c0 + m) * P]
                    )
                    maskt_sl = lp.tile([P, m, P], BF16, tag="maskt")
                    nc.sync.dma_start(
                        out=maskt_sl[:], in_=maskth[:, c0 * P : (c0 + m) * P]
                    )
                    # er for this slot's 128 dst nodes via folded weight
                    fown = sm1.tile([P, P], BF16, tag="fown")
                    nc.sync.dma_start(
                        out=fown[:], in_=featT[:, j * P : (j + 1) * P]
                    )
                    erps = pzp.tile([P, H1], F32, tag="aux")
                    nc.tensor.matmul(
                        out=erps[:], lhsT=fown[:], rhs=wer1_s[:],
                        start=True, stop=True,
                    )
                    ertile = sm1.tile([P, H1], BF16, tag="ertile")
                    nc.vector.tensor_copy(out=ertile[:], in_=erps[:])
                    acc = pac.tile([P, W1A], F32, tag="accz")
                    for b0 in range(0, m, ZB):
                        nb = min(ZB, m - b0)
                        zs = sm1.tile([P, ZB, W1A], BF16, tag="zs")
                        zcp = sm1.tile([P, ZB, HD1], BF16, tag="zcp")
                        exm = sm1.tile([P, ZB * H1], F32, tag="exm")
                        exm2 = sm1.tile([P, ZB * H1], F32, tag="exm2")
                        zep = pzp.tile([P, ZB, HD1], F32, tag="zep")
                        elp = pzp.tile([P, ZB, H1], F32, tag="aux")
                        for c in range(nb):
                            nc.tensor.matmul(
                                out=zep[:, c, :],
                                lhsT=feT_sl[:, b0 + c, :],
                                rhs=w1_s[:],
                                start=True, stop=True,
                            )
                            nc.tensor.matmul(
                                out=elp[:, c, :],
                                lhsT=feT_sl[:, b0 + c, :],
                                rhs=w1aug_s[:, HD1:W1A],
                                start=True, stop=False,
                            )
                            nc.tensor.matmul(
                                out=elp[:, c, :],
                                lhsT=maskt_sl[:, b0 + c, :],
                                rhs=ertile[:],
                                start=False, stop=True,
                                skip_group_check=True,
                            )
                        # PSUM z -> SBUF bf16 on ACT engine (DVE diet)
                        nc.scalar.copy(
                            out=zcp[:, 0:nb, :], in_=zep[:, 0:nb, :]
                        )
                        ev = elp[:, 0:nb, :]
                        nc.scalar.activation(
                            out=exm[:, 0 : nb * H1].rearrange(
                                "p (b h) -> p b h", h=H1),
                            in_=ev, func=mybir.ActivationFunctionType.Exp,
                        )
                        nc.scalar.activation(
                            out=exm2[:, 0 : nb * H1].rearrange(
                                "p (b h) -> p b h", h=H1),
                            in_=ev, func=mybir.ActivationFunctionType.Exp,
                            scale=NEG_SLOPE,
                        )
                        nc.vector.tensor_tensor(
                            out=zs[:, 0:nb, HD1:W1A],
                            in0=exm[:, 0 : nb * H1].rearrange(
                                "p (b h) -> p b h", h=H1),
                            in1=exm2[:, 0 : nb * H1].rearrange(
                                "p (b h) -> p b h", h=H1),
                            op=mybir.AluOpType.max,
                        )
                        nc.vector.tensor_tensor(
                            out=zs[:, 0:nb, 0:HD1].rearrange(
                                "p b (h d) -> p b h d", d=HID),
                            in0=zcp[:, 0:nb, :].rearrange(
                                "p b (h d) -> p b h d", d=HID),
                            in1=zs[:, 0:nb, HD1:W1A]
                            .rearrange("p b h -> p b h")
                            .unsqueeze(3)
                            .to_broadcast([P, nb, H1, HID]),
                            op=mybir.AluOpType.mult,
                        )
                        for c in range(nb):
                            nc.tensor.matmul(
                                out=acc[:],
                                lhsT=mask_sl[:, b0 + c, :],
                                rhs=zs[:, c, :],
                                start=(b0 + c == 0),
                                stop=(b0 + c == m - 1),
                            )
                    # epilogue: h = elu(numer/denom + b1)
                    dpl = sm1.tile([P, H1], F32, tag="dpl")
                    nc.vector.tensor_scalar_add(dpl[:], acc[:, HD1:W1A], EPS)
                    rec = sm1.tile([P, H1], F32, tag="rec")
                    nc.vector.reciprocal(rec[:], dpl[:])
                    x = lp.tile([P, HD1], F32, tag="x")
                    nc.vector.tensor_tensor(
                        out=x[:].rearrange("p (h d) -> p h d", d=HID),
                        in0=acc[:, 0:HD1].rearrange("p (h d) -> p h d", d=HID),
                        in1=rec[:].unsqueeze(2).to_broadcast([P, H1, HID]),
                        op=mybir.AluOpType.mult,
                    )
                    nc.vector.tensor_tensor(
                        out=x[:], in0=x[:], in1=b1_s[:], op=mybir.AluOpType.add
                    )
                    uexp = lp.tile([P, HD1], F32, tag="uexp")
                    nc.scalar.activation(
                        out=uexp[:], in_=x[:],
                        func=mybir.ActivationFunctionType.Exp,
                    )
                    umin = lp.tile([P, HD1], F32, tag="umin")
                    nc.vector.tensor_scalar(
                        umin[:], uexp[:], 1.0, -1.0,
                        op0=mybir.AluOpType.min, op1=mybir.AluOpType.add,
                    )
                    h = lp.tile([P, HD1], F32, tag="h")
                    nc.vector.tensor_scalar_max(h[:], x[:], 0.0)
                    nc.vector.tensor_tensor(
                        out=h[:], in0=h[:], in1=umin[:], op=mybir.AluOpType.add
                    )
                    # layer-2 projection: z2 = h @ W2, el2/er2 scores
                    z2ps = pac.tile([P, OUT], F32, tag="accz")
                    for half in range(2):
                        htp = pzp.tile([P, P], F32, tag="aux")
                        nc.tensor.transpose(
                            out=htp[:], in_=h[:, half * P : (half + 1) * P],
                            identity=ident[:],
                        )
                        hts = sm1.tile([P, P], F32, tag="hts")
                        nc.scalar.copy(out=hts[:], in_=htp[:])
                        nc.tensor.matmul(
                            out=z2ps[:], lhsT=hts[:],
                            rhs=(w2a_s[:] if half == 0 else w2b_s[:]),
                            start=(half == 0), stop=(half == 1),
                        )
                    z2row = sm1.tile([P, Z2ROW], BF16, tag="z2row")
                    nc.vector.tensor_copy(out=z2row[:, 0:OUT], in_=z2ps[:])
                    tmp2 = sm1.tile([P, OUT], F32, tag="tmp2")
                    nc.vector.tensor_tensor(
                        out=tmp2[:], in0=z2ps[:], in1=al2_s[:],
                        op=mybir.AluOpType.mult,
                    )
                    nc.vector.reduce_sum(
                        out=z2row[:, 40:42].bitcast(F32),
                        in_=tmp2[:].rearrange("p (a d) -> p a d", a=1),
                        axis=mybir.AxisListType.X,
                    )
                    nc.vector.tensor_tensor(
                        out=tmp2[:], in0=z2ps[:], in1=ar2_s[:],
                        op=mybir.AluOpType.mult,
                    )
                    with nc.allow_low_precision(reason="er2 is bf16 by design"):
                        nc.vector.reduce_sum(
                            out=z2row[:, 42:43],
                            in_=tmp2[:].rearrange("p (a d) -> p a d", a=1),
                            axis=mybir.AxisListType.X,
                        )
                    nc.vector.tensor_copy(
                        out=er2all[:, j : j + 1], in_=z2row[:, 42:43]
                    )
                    jg = int(g_of_slot_py[j])
                    jr = j - int(GSTART[jg])
                    nc.sync.dma_start(
                        out=z2locs[jg][jr * P : (jr + 1) * P, :],
                        in_=z2row[:],
                    )

                    # ---- pipelined AllGather + gathers + consumption ----
                    if g_next < N_G and j == int(GSTART[g_next + 1]) - 1:
                        g = g_next
                        nc.gpsimd.collective_compute(
                            "AllGather",
                            mybir.AluOpType.bypass,
                            ins=[z2locs[g][:]],
                            outs=[z2tabs[g][:]],
                            replica_groups=[list(range(N_CORES))],
                        )
                        for p in pieces:
                            if p["g"] == g:
                                heapq.heappush(
                                    gather_heap,
                                    (p["j_last"], g, p["pc"]),
                                )
                        g_next += 1
                    pump(j)

                # drain
                guard = 0
                while gather_heap or gathered:
                    pump(TPC, consume_budget=4, drain=True)
                    guard += 1
                    assert guard < 1000

                # ---------- final epilogue per slot ----------
                for j in range(TPC):
                    rec2 = sm2.tile([P, 1], F32, tag="rec2")
                    dpl2 = sm2.tile([P, 1], F32, tag="dpl2")
                    nc.vector.tensor_scalar_add(
                        dpl2[:], acc3[:, j, OUT : OUT + 1], EPS
                    )
                    nc.vector.reciprocal(rec2[:], dpl2[:])
                    ot = sm2.tile([P, OUT], F32, tag="ot")
                    nc.scalar.mul(ot[:], acc3[:, j, 0:OUT], rec2[:, 0:1])
                    nc.vector.tensor_tensor(
                        out=ot[:], in0=ot[:], in1=b2_s[:],
                        op=mybir.AluOpType.add,
                    )
                    nc.sync.dma_start(
                        out=out_d[j * P : (j + 1) * P, :], in_=ot[:]
                    )

    nc.compile()
    return nc


# --------------------------------------------------------------------------
# Entry point
# --------------------------------------------------------------------------
def _prep_inputs(feat, W1, al1, ar1, b1, W2, al2, ar2, b2, plan):
    pi = plan["pi"]
    M = plan["M"]
    feat_pad = np.zeros((N_PAD + 1, IN_SIZE), np.float32)
    feat_pad[:N_NODES] = np.asarray(feat, np.float32)
    node_at = np.empty(N_PAD, np.int64)
    node_at[pi] = np.arange(N_PAD)

    W1 = np.asarray(W1, np.float32)
    al1 = np.asarray(al1, np.float32).reshape(H1, HID)
    ar1 = np.asarray(ar1, np.float32).reshape(H1, HID)
    w_el = (W1.reshape(IN_SIZE, H1, HID) * al1[None]).sum(-1)  # [128, 8]
    w_er = (W1.reshape(IN_SIZE, H1, HID) * ar1[None]).sum(-1)  # [128, 8]
    w1aug = np.concatenate([W1, w_el], axis=1)  # [128, 264]
    W2 = np.asarray(W2, np.float32)

    def rep(v, w):
        return np.broadcast_to(
            np.asarray(v, np.float32).reshape(1, w), (P, w)
        ).copy()

    common = {
        "w1aug": w1aug.astype(NP_BF16),
        "w1": W1.astype(NP_BF16),
        "wer1": w_er.astype(NP_BF16),
        "b1m": rep(b1, HD1),
        "w2a": W2[:P].copy(),
        "w2b": W2[P:].copy(),
        "al2m": rep(al2, OUT),
        "ar2m": rep(ar2, OUT),
        "b2m": rep(b2, OUT),
    }
    iota = np.arange(P, dtype=np.int64)
    touches = plan["touches"]
    tot_chunks = len(touches)
    touch_base = np.zeros(tot_chunks + 1, np.int64)
    for c in range(tot_chunks):
        touch_base[c + 1] = touch_base[c] + len(touches[c])
    n_touch = int(touch_base[-1])

    in_maps = []
    for k in range(N_CORES):
        im = dict(common)
        im["featT"] = (
            feat_pad[node_at[k * NPC : (k + 1) * NPC]].T.astype(NP_BF16)
        )
        src_e = plan["src_e"][k]
        row_e = plan["row_e"][k]
        im["feT"] = np.ascontiguousarray(
            feat_pad[src_e].T.astype(NP_BF16)
        )
        re2 = row_e.reshape(M, P)
        mask = (re2[:, None, :] == iota[None, :, None])  # [M, 128d, 128e]
        im["maskh"] = np.ascontiguousarray(
            mask.transpose(2, 0, 1).reshape(P, M * P).astype(NP_BF16)
        )
        im["maskth"] = np.ascontiguousarray(
            mask.transpose(1, 0, 2).reshape(P, M * P).astype(NP_BF16)
        )
        im["zidx"] = _pack_idxs(plan["zidx"][k])
        row2 = plan["row2"][k]
        r2c = row2.reshape(tot_chunks, P)
        m2h = np.zeros((n_touch, P, P), np.bool_)  # [touch, 128d, 128e]
        for c in range(tot_chunks):
            for ti, (j, a, b) in enumerate(touches[c]):
                tb = int(touch_base[c]) + ti
                rr = r2c[c, a:b]
                valid = rr >= 0
                cols = np.arange(a, b)[valid]
                m2h[tb, rr[valid], cols] = True
        im["mask2h"] = np.ascontiguousarray(
            m2h.transpose(2, 0, 1).reshape(P, n_touch * P).astype(NP_BF16)
        )
        im["mask2t"] = np.ascontiguousarray(
            m2h.transpose(1, 0, 2).reshape(P, n_touch * P).astype(NP_BF16)
        )
        in_maps.append(im)
    return in_maps


_CACHE = {}


def kernel(feat, src, dst, W1, al1, ar1, b1, W2, al2, ar2, b2, _trace=False):
    plan = build_plan(src, dst)
    key = (
        tuple(plan["m_j"]),
        tuple(np.asarray(plan["seg_len"]).ravel().tolist()),
    )
    if key not in _CACHE:
        _CACHE[key] = build_program(plan)
    nc = _CACHE[key]
    in_maps = _prep_inputs(feat, W1, al1, ar1, b1, W2, al2, ar2, b2, plan)
    res = run_bass_kernel_spmd(
        nc, in_maps, core_ids=list(range(N_CORES)), trace=_trace
    )
    outs = np.concatenate(
        [np.asarray(r["out"], np.float32) for r in res.results], axis=0
    )
    full = outs[plan["pi"]][:N_NODES]
    if _trace:
        kernel.last_exec_time_ns = res.exec_time_ns
        kernel.last_results = res
    return np.ascontiguousarray(full.astype(np.float32))
